# revision 16
# baseline (speedup 1.0000x reference)
"""DGANet dual-GAT layer on 8 Trainium2 NeuronCores (Bass/Tile).

Math (per branch b in {n, d}):
    Wh = h @ W_b                                  [4096, 256]
    e  = leaky_relu(s1_i + s2_j, 0.2)             s1 = h@(W@a1), s2 = h@(W@a2)
    att = softmax(where(adj>0, e, -9e15), axis=-1)
    f_b = elu(att @ Wh)
Output: f_n + f_d.

Sharding: 1D row-parallel over the 4096 attention rows (512 rows/core).
Each core holds its score block transposed, P^T[j, i] (j on partitions), so
the att @ Wh contraction over j runs directly on the tensor engine with the
locally-computed full Wh[j, f] as the stationary operand.  The adjacency
mask is folded into the logits as a host-prepared additive bias
(0 or -16384, bf16): exp underflows masked entries to exactly 0, which also
makes the softmax denominator a ones-column matmul over P^T.

Matmul dtype: float32r (19-bit tf32-like operand rounding, full-rate PE
streaming) when USE_F32R, else plain fp32 (bit-exact, quarter-rate PE).
"""

from contextlib import ExitStack

import numpy as np
import ml_dtypes

import concourse.bass as bass
import concourse.bacc as bacc
import concourse.mybir as mybir
import concourse.tile as tile
from concourse import bass_utils
from concourse.masks import make_identity

N, FIN, F = 4096, 512, 256
NCORES = 8
R = N // NCORES            # 512 attention rows per core
P = 128                    # partitions
NJT = N // P               # 32 j-tiles
NKT = FIN // P             # 4 fin contraction tiles
WC = F + 2                 # rhs_aug cols: [W | W@a1 | W@a2] (even N for fp32r)
MASKB = -16384.0           # additive mask: exp(x - 16384) == 0.0 in fp32
ALPHA = 0.2

USE_F32R = True

F32 = mybir.dt.float32
F32R = mybir.dt.float32r
BF16 = mybir.dt.bfloat16
AF = mybir.ActivationFunctionType
ALU = mybir.AluOpType
BR = ("n", "d")
DT_MM = F32R if USE_F32R else F32


def build_program(reps=None):
    """reps=None: single-shot program (grading path).  reps=K: body wrapped
    in a K-iteration hardware loop, for wall-clock HW timing by slope."""
    nc = bacc.Bacc("TRN2", target_bir_lowering=False, debug=False,
                   num_devices=NCORES)

    hT = nc.dram_tensor("ht", [FIN, N], F32, kind="ExternalInput").ap()
    hTo = nc.dram_tensor("hto", [FIN, R], F32, kind="ExternalInput").ap()
    W = {b: nc.dram_tensor(f"w_{b}", [FIN, F], F32, kind="ExternalInput").ap()
         for b in BR}
    WT = {b: nc.dram_tensor(f"wt_{b}", [F, FIN], F32, kind="ExternalInput").ap()
          for b in BR}
    A1 = {b: nc.dram_tensor(f"a1_{b}", [F, 1], F32, kind="ExternalInput").ap()
          for b in BR}
    A2 = {b: nc.dram_tensor(f"a2_{b}", [F, 1], F32, kind="ExternalInput").ap()
          for b in BR}
    MT = {b: nc.dram_tensor(f"mt_{b}", [N, R], BF16, kind="ExternalInput").ap()
          for b in BR}
    OUT = nc.dram_tensor("out", [R, F], F32, kind="ExternalOutput").ap()

    with tile.TileContext(nc) as tc:
        if reps is None:
            with ExitStack() as ctx:
                _body(ctx, nc, tc, hT, hTo, W, WT, A1, A2, MT, OUT)
        else:
            with tc.For_i(0, reps, 1,
                          hint_engines=(mybir.EngineType.PE,)):
                with ExitStack() as ctx:
                    _body(ctx, nc, tc, hT, hTo, W, WT, A1, A2, MT, OUT)
    nc.compile()
    return nc


def _body(ctx, nc, tc, hT, hTo, W, WT, A1, A2, MT, OUT):
    CH = 4                      # hT column chunks (DMA pipelining granularity)
    CHW = N // CH               # 1024 cols per chunk

    consts = ctx.enter_context(tc.tile_pool(name="consts", bufs=1))
    rawp = ctx.enter_context(tc.tile_pool(name="rawp", bufs=3))
    # All pp_work tiles share one tag: 2 PSUM banks total, recycled.
    pp_work = ctx.enter_context(tc.tile_pool(name="pp_work", bufs=2,
                                             space="PSUM"))
    # 6 distinct single-buf accumulators: 6 PSUM banks (8 total with pp_work).
    pp_acc = ctx.enter_context(tc.tile_pool(name="pp_acc", bufs=1,
                                            space="PSUM"))
    whp = ctx.enter_context(tc.tile_pool(name="whp", bufs=4))
    maskp = ctx.enter_context(tc.tile_pool(name="maskp", bufs=6))
    workp = ctx.enter_context(tc.tile_pool(name="workp", bufs=4))
    pexp = ctx.enter_context(tc.tile_pool(name="pexp", bufs=3))
    epip = ctx.enter_context(tc.tile_pool(name="epip", bufs=2))

    ident = consts.tile([P, P], F32, tag="ident")
    make_identity(nc, ident)
    onesf = consts.tile([P, P], F32, tag="onesf")
    nc.vector.memset(onesf, 1.0)
    ones_mat = consts.tile([P, P], DT_MM, tag="ones_mat")
    nc.vector.tensor_copy(out=ones_mat, in_=onesf)

    # ---- stage 0: small weights in SBUF, wa = W@a on PE -------------------
    wt_sb = {}
    a_sb = {}
    for b in BR:
        for fk in range(2):
            t = consts.tile([P, FIN], F32, tag=f"wt_{b}{fk}")
            nc.sync.dma_start(out=t, in_=WT[b][fk * P:(fk + 1) * P, :])
            wt_sb[b, fk] = t
        for fk in range(2):
            t = consts.tile([P, 2], F32, tag=f"a12_{b}{fk}")
            nc.sync.dma_start(out=t[:, 0:1],
                              in_=A1[b][fk * P:(fk + 1) * P, :])
            nc.sync.dma_start(out=t[:, 1:2],
                              in_=A2[b][fk * P:(fk + 1) * P, :])
            a_sb[b, fk] = t

    # wa[b] chunks in psum [128, 2*NKT]: cols 0..3 = W@a1, 4..7 = W@a2
    wa_sb = {}
    wa_r = {}
    for b in BR:
        ps = pp_work.tile([P, 2 * NKT], F32, tag="pswork")
        for m in range(NKT):
            for fk in range(2):
                nc.tensor.matmul(
                    ps[:, 2 * m:2 * m + 2],
                    lhsT=wt_sb[b, fk][:, m * P:(m + 1) * P],
                    rhs=a_sb[b, fk],
                    start=(fk == 0), stop=(fk == 1))
        t = consts.tile([P, 2 * NKT], F32, tag=f"wa_{b}")
        nc.vector.tensor_copy(out=t, in_=ps)
        wa_sb[b] = t
        # wa1 chunk m replicated across 128 cols: stationary operand whose
        # matmul output is s1 already broadcast over partitions.
        reps = []
        for m in range(NKT):
            r = consts.tile([P, P], DT_MM, tag=f"war_{b}{m}", name=f"war{m}")
            nc.vector.tensor_copy(
                out=r, in_=t[:, 2 * m:2 * m + 1].broadcast_to((P, P)))
            reps.append(r)
        wa_r[b] = reps

    # rhs_aug[b][k] = [W rows k*128.. | (W@a2) chunk k]  -> [128, 257]
    rhs_aug = {}
    for b in BR:
        for k in range(NKT):
            t = rawp.tile([P, WC], F32, tag="augraw")
            nc.sync.dma_start(out=t[:, 0:F], in_=W[b][k * P:(k + 1) * P, :])
            nc.vector.tensor_copy(
                out=t[:, F:F + 2], in_=wa_sb[b][:, 2 * k:2 * k + 2])
            tr = consts.tile([P, WC], DT_MM, tag=f"aug_{b}{k}")
            nc.vector.tensor_copy(out=tr, in_=t)
            rhs_aug[b, k] = tr

    # ---- own-row h block + s1 row vectors ---------------------------------
    hto_sb = []
    for k in range(NKT):
        raw = rawp.tile([P, R], F32, tag="htoraw")
        nc.sync.dma_start(out=raw, in_=hTo[k * P:(k + 1) * P, :])
        t = consts.tile([P, R], DT_MM, tag=f"hto{k}")
        nc.vector.tensor_copy(out=t, in_=raw)
        hto_sb.append(t)

    s1b = {}
    for b in BR:
        ps1 = pp_work.tile([P, R], F32, tag="pswork")
        for k in range(NKT):
            nc.tensor.matmul(
                ps1, lhsT=wa_r[b][k], rhs=hto_sb[k],
                start=(k == 0), stop=(k == NKT - 1))
        t = consts.tile([P, R], F32, tag=f"s1b_{b}")
        nc.scalar.copy(out=t, in_=ps1)
        s1b[b] = t

    # ---- fused main loop: Wh tile then its attention work, per (jt, b) ----
    ht_sb = {}
    for k in range(NKT):
        for ch in range(CH):
            raw = rawp.tile([P, CHW], F32, tag="htraw")
            nc.sync.dma_start(
                out=raw, in_=hT[k * P:(k + 1) * P, ch * CHW:(ch + 1) * CHW])
            t = consts.tile([P, CHW], DT_MM, tag=f"ht{k}_{ch}")
            if (k + ch) % 2 == 0:
                nc.vector.tensor_copy(out=t, in_=raw)
            else:
                nc.scalar.copy(out=t, in_=raw)
            ht_sb[k, ch] = t

    acc = {}
    for b in BR:
        acc[b, 0] = pp_acc.tile([P, R], F32, tag=f"acc_{b}0", name=f"acc_{b}0")
        acc[b, 1] = pp_acc.tile([P, R], F32, tag=f"acc_{b}1", name=f"acc_{b}1")
        acc[b, "rs"] = pp_acc.tile([P, R], F32, tag=f"acc_{b}rs",
                                   name=f"acc_{b}rs")

    for jt in range(NJT):
        ch, off = divmod(jt * P, CHW)
        for bi, b in enumerate(BR):
            par = (jt + bi) % 2
            ps = pp_work.tile([P, WC], F32, tag="pswork")
            for k in range(NKT):
                nc.tensor.matmul(
                    ps, lhsT=ht_sb[k, ch][:, off:off + P],
                    rhs=rhs_aug[b, k],
                    start=(k == 0), stop=(k == NKT - 1))
            wh = whp.tile([P, F], DT_MM, tag="wh")
            s2 = workp.tile([P, 1], F32, tag="s2")
            if par == 0:
                nc.scalar.copy(out=wh, in_=ps[:, 0:F])
                nc.vector.tensor_copy(out=s2, in_=ps[:, F + 1:F + 2])
            else:
                nc.vector.tensor_copy(out=wh, in_=ps[:, 0:F])
                nc.scalar.copy(out=s2, in_=ps[:, F + 1:F + 2])

            m = maskp.tile([P, R], BF16, tag="mask")
            nc.sync.dma_start(out=m, in_=MT[b][jt * P:(jt + 1) * P, :])
            u = workp.tile([P, R], F32, tag="u")
            nc.vector.scalar_tensor_tensor(
                out=u, in0=s1b[b], scalar=s2, in1=m,
                op0=ALU.add, op1=ALU.add)
            lr = workp.tile([P, R], F32, tag="lr")
            nc.vector.scalar_tensor_tensor(
                out=lr, in0=u, scalar=ALPHA, in1=u,
                op0=ALU.mult, op1=ALU.max)
            pt = pexp.tile([P, R], DT_MM, tag="pt")
            nc.scalar.activation(out=pt, in_=lr, func=AF.Exp)
            first, last = (jt == 0), (jt == NJT - 1)
            nc.tensor.matmul(acc[b, 0], lhsT=wh[:, 0:P], rhs=pt,
                             start=first, stop=last)
            nc.tensor.matmul(acc[b, 1], lhsT=wh[:, P:F], rhs=pt,
                             start=first, stop=last)
            nc.tensor.matmul(acc[b, "rs"], lhsT=ones_mat, rhs=pt,
                             start=first, stop=last)

    # ---- epilogue: divide, elu, combine branches, transpose out -----------
    rb = {}
    for b in BR:
        t = epip.tile([P, R], F32, tag=f"rb_{b}")
        nc.vector.reciprocal(out=t, in_=acc[b, "rs"])
        rb[b] = t

    comb = []
    for fh in range(2):
        tb = {}
        for b in BR:
            o = epip.tile([P, R], F32, tag="o")
            nc.vector.scalar_tensor_tensor(
                out=o, in0=acc[b, fh], scalar=1.0, in1=rb[b],
                op0=ALU.mult, op1=ALU.mult)
            rl = epip.tile([P, R], F32, tag="rl")
            nc.scalar.activation(out=rl, in_=o, func=AF.Relu)
            em = epip.tile([P, R], F32, tag="em")
            nc.scalar.activation(out=em, in_=o, func=AF.Exp)
            t = epip.tile([P, R], F32, tag="t")
            # t = min(exp(o), 1) + relu(o)  ==  elu(o) + 1
            nc.vector.scalar_tensor_tensor(
                out=t, in0=em, scalar=1.0, in1=rl, op0=ALU.min, op1=ALU.add)
            tb[b] = t
        c = epip.tile([P, R], F32, tag="comb")
        # c = (t_n - 2) + t_d  ==  elu(o_n) + elu(o_d)
        nc.vector.scalar_tensor_tensor(
            out=c, in0=tb["n"], scalar=-2.0, in1=tb["d"],
            op0=ALU.add, op1=ALU.add)
        comb.append(c)

    for it in range(R // P):
        ps = pp_work.tile([P, F], F32, tag="pswork")
        for fh in range(2):
            nc.tensor.transpose(
                ps[:, fh * P:(fh + 1) * P],
                comb[fh][:, it * P:(it + 1) * P], ident)
        o = epip.tile([P, F], F32, tag="oout")
        nc.vector.tensor_copy(out=o, in_=ps)
        nc.sync.dma_start(out=OUT[it * P:(it + 1) * P, :], in_=o)


_CACHED = None


def _get_program():
    global _CACHED
    if _CACHED is None:
        _CACHED = build_program()
    return _CACHED


def _prep_inputs(h, adj_n, adj_d, W_n, a1_n, a2_n, W_d, a1_d, a2_d):
    h = np.asarray(h, np.float32)
    hT = np.ascontiguousarray(h.T)
    com = {
        "ht": hT,
        "w_n": np.asarray(W_n, np.float32),
        "w_d": np.asarray(W_d, np.float32),
        "wt_n": np.ascontiguousarray(np.asarray(W_n, np.float32).T),
        "wt_d": np.ascontiguousarray(np.asarray(W_d, np.float32).T),
        "a1_n": np.asarray(a1_n, np.float32),
        "a2_n": np.asarray(a2_n, np.float32),
        "a1_d": np.asarray(a1_d, np.float32),
        "a2_d": np.asarray(a2_d, np.float32),
    }
    adj = {"n": np.asarray(adj_n), "d": np.asarray(adj_d)}
    maps = []
    for c in range(NCORES):
        m = dict(com)
        m["hto"] = np.ascontiguousarray(hT[:, c * R:(c + 1) * R])
        for b in BR:
            blk = adj[b][c * R:(c + 1) * R, :]          # [R, N]
            mt = np.where(blk.T > 0, np.float32(0.0), np.float32(MASKB))
            m[f"mt_{b}"] = mt.astype(ml_dtypes.bfloat16)
        maps.append(m)
    return maps


def run_on_hw(inputs, trace=False):
    nc = _get_program()
    maps = _prep_inputs(
        inputs["h"], inputs["adj_n"], inputs["adj_d"],
        inputs["W_n"], inputs["a1_n"], inputs["a2_n"],
        inputs["W_d"], inputs["a1_d"], inputs["a2_d"])
    res = bass_utils.run_bass_kernel_spmd(
        nc, maps, core_ids=list(range(NCORES)), trace=trace)
    out = np.concatenate([res.results[c]["out"] for c in range(NCORES)],
                         axis=0)
    return out, res


def kernel(**inputs):
    out, _ = run_on_hw(inputs, trace=False)
    return out
